# revision 9
# baseline (speedup 1.0000x reference)
"""GQA attention block (QKV proj + causal attention + output proj) on 8 trn2 cores.

Sharding: core c -> (batch b = c//4, kv-group g = c%4). Each core computes 4 Q
heads (one KV-head group) of one batch and a partial o_proj output; the host
sums the 4 partials per batch (row-sharded o_proj all-reduce done host-side).

All device matmuls are fp32 (exact). Attention uses transposed scores
S^T[tk, tq] so the softmax denominator comes for free from a ones-column
appended to V, and no on-chip transposes of attention weights are needed.
"""

import math

import numpy as np

# Model dims (hardcoded per contract; kernel.py must be self-contained).
B = 2
T = 2048
E = 2048
HD = 128               # head dim
NH = 16                # query heads total
NKV = 4                # kv heads total
NHC = 4                # query heads per core
P = 128
KO = E // P            # 16 contraction subtiles of 128
NQUART = 4             # xT streamed in 4 e-quarters of 512
TQC = T // 512         # 4 query chunks of 512
TB = T // P            # 16 t blocks of 128
SCALE = 1.0 / math.sqrt(HD)
N_CORES = 8

_NC_CACHE = {}


def _build_nc():
    import concourse.bacc as bacc
    import concourse.mybir as mybir
    import concourse.tile as tile
    from concourse.masks import make_identity, make_upper_triangular

    f32 = mybir.dt.float32
    nc = bacc.Bacc(None, target_bir_lowering=False)

    xT = nc.dram_tensor("xT", [E, T], f32, kind="ExternalInput")
    wqT = nc.dram_tensor("wqT", [E, NHC * HD], f32, kind="ExternalInput")
    wkT = nc.dram_tensor("wkT", [E, HD], f32, kind="ExternalInput")
    wvT = nc.dram_tensor("wvT", [E, HD], f32, kind="ExternalInput")
    woT = nc.dram_tensor("woT", [NHC * HD, E], f32, kind="ExternalInput")
    out = nc.dram_tensor("out", [T, E], f32, kind="ExternalOutput")

    xT_r = xT.rearrange("(ko p) t -> p ko t", p=P)        # [128, 16, T]
    wqT_r = wqT.rearrange("(ko p) d -> p ko d", p=P)      # [128, 16, 512]
    wkT_r = wkT.rearrange("(ko p) d -> p ko d", p=P)      # [128, 16, 128]
    wvT_r = wvT.rearrange("(ko p) d -> p ko d", p=P)
    woT_r = woT.rearrange("(h p) e -> p h e", p=P)        # [128, 4, E]
    out_r = out.rearrange("(tb p) e -> p tb e", p=P)      # [128, 16, E]

    with tile.TileContext(nc) as tc:
        with (
            tc.tile_pool(name="const", bufs=1) as constp,
            tc.tile_pool(name="qkv", bufs=1) as qkvp,
            tc.tile_pool(name="ps_acc", bufs=2, space="PSUM") as ps_acc,
            tc.tile_pool(name="ps_y", bufs=4, space="PSUM") as ps_y,
            tc.tile_pool(name="ps_t", bufs=2, space="PSUM") as ps_t,
        ):
            identity = constp.tile([P, P], f32, tag="ident")
            make_identity(nc, identity)

            # tri[p, q] = 1.0 where p <= q — causal mask for the one
            # tk==tq diagonal 128x128 sub-block.
            tri = constp.tile([P, P], f32, tag="tri")
            make_upper_triangular(nc, tri[:], val=1.0, diag=True)

            QT = qkvp.tile([P, NHC, T], f32, tag="QT")     # q^T per head [d, t]
            KT = qkvp.tile([P, T], f32, tag="KT")          # k^T [d, t]
            VT = qkvp.tile([P, T], f32, tag="VT")          # v^T [d, t]
            VAUG = qkvp.tile([P, TB, HD + 1], f32, tag="VAUG")  # v blocks [tk, 129]

            # ---- Phase 1: projections. q^T/k^T/v^T = W @ x^T, contracting
            # over e; xT streamed in 4 e-quarters, weights resident.
            with (
                tc.tile_pool(name="w1", bufs=1) as w1p,
                tc.tile_pool(name="xq", bufs=2) as xqp,
            ):
                WQT = w1p.tile([P, KO, NHC * HD], f32, tag="WQT")
                WKT = w1p.tile([P, KO, HD], f32, tag="WKT")
                WVT = w1p.tile([P, KO, HD], f32, tag="WVT")

                for q in range(NQUART):
                    xt = xqp.tile([P, 4, T], f32, tag="xq")
                    # Interleave activations and weights per e-subtile so the
                    # first matmuls aren't stuck behind the full weight load.
                    for eo in range(4):
                        ko = 4 * q + eo
                        nc.sync.dma_start(xt[:, eo], xT_r[:, ko])
                        nc.sync.dma_start(WQT[:, ko], wqT_r[:, ko])
                        nc.sync.dma_start(WKT[:, ko], wkT_r[:, ko])
                        nc.sync.dma_start(WVT[:, ko], wvT_r[:, ko])

                    def _acc(dst, lhsT_of_eo, tcol):
                        ps = ps_acc.tile([P, 512], f32, tag="ps_acc")
                        for eo in range(4):
                            nc.tensor.matmul(
                                ps[:],
                                lhsT_of_eo(eo),
                                xt[:, eo, tcol * 512:(tcol + 1) * 512],
                                start=(eo == 0),
                                stop=(eo == 3),
                            )
                        if q == 0:
                            nc.vector.tensor_copy(dst, ps[:])
                        else:
                            nc.vector.tensor_add(out=dst, in0=dst, in1=ps[:])

                    for h in range(NHC):
                        for tcol in range(TQC):
                            _acc(
                                QT[:, h, tcol * 512:(tcol + 1) * 512],
                                lambda eo, h=h: WQT[:, 4 * q + eo, h * HD:(h + 1) * HD],
                                tcol,
                            )
                    for tcol in range(TQC):
                        _acc(
                            KT[:, tcol * 512:(tcol + 1) * 512],
                            lambda eo: WKT[:, 4 * q + eo],
                            tcol,
                        )
                    for tcol in range(TQC):
                        _acc(
                            VT[:, tcol * 512:(tcol + 1) * 512],
                            lambda eo: WVT[:, 4 * q + eo],
                            tcol,
                        )

            # v^T -> v natural layout blocks, with ones column for the
            # softmax denominator.
            nc.vector.memset(VAUG[:, :, HD:HD + 1], 1.0)
            for tb in range(TB):
                pst = ps_t.tile([P, P], f32, tag="ps_t")
                nc.tensor.transpose(pst[:], VT[:, tb * P:(tb + 1) * P], identity[:])
                nc.vector.tensor_copy(VAUG[:, tb, 0:HD], pst[:])

            # ---- Phases 2+3 pools
            with (
                tc.tile_pool(name="big2", bufs=1) as big2,
                tc.tile_pool(name="work", bufs=4) as work,
                tc.tile_pool(name="owork", bufs=3) as owork,
            ):
                YT = big2.tile([P, NHC, T], f32, tag="YT")   # y^T per head [d, t]
                WOT = big2.tile([P, NHC, E], f32, tag="WOT")
                for ko in range(4):
                    nc.sync.dma_start(
                        WOT[:, ko], woT_r[:, ko]
                    )

                # ---- Phase 2: causal attention, transposed scores. For
                # diagonal-region tk blocks the score matmul is narrowed to
                # the causally-valid tq columns; only the single tk==tq
                # 128x128 sub-block needs the triangular mask.
                for h in range(NHC):
                    for tqc in range(TQC):
                        ntk = 4 * (tqc + 1)   # tk blocks up to the diagonal
                        psy = [
                            ps_y.tile([P, HD + 1], f32, tag="ps_y", name=f"psy_{j}")
                            for j in range(4)
                        ]
                        for tk in range(ntk):
                            i = tk - 4 * tqc  # >= 0 inside the diagonal region
                            off = max(0, i) * P   # local tq offset of valid cols
                            w = 512 - off
                            pss = ps_acc.tile([P, 512], f32, tag="ps_acc")
                            nc.tensor.matmul(
                                pss[:, 0:w],
                                KT[:, tk * P:(tk + 1) * P],
                                QT[:, h, tqc * 512 + off:(tqc + 1) * 512],
                                start=True,
                                stop=True,
                            )
                            es = work.tile([P, 512], f32, tag="expS")
                            nc.scalar.activation(
                                es[:, 0:w], pss[:, 0:w],
                                mybir.ActivationFunctionType.Exp,
                                scale=SCALE,
                            )
                            if i >= 0:
                                nc.vector.tensor_mul(
                                    out=es[:, 0:P], in0=es[:, 0:P], in1=tri[:]
                                )
                            for j in range(max(0, i), 4):
                                nc.tensor.matmul(
                                    psy[j][:],
                                    es[:, j * P - off:(j + 1) * P - off],
                                    VAUG[:, tk],
                                    start=(tk == 0),
                                    stop=(tk == 4 * tqc + j),
                                )
                        for j in range(4):
                            jg = 4 * tqc + j
                            recip = work.tile([P, 1], f32, tag="recip")
                            nc.vector.reciprocal(recip[:], psy[j][:, HD:HD + 1])
                            ysb = work.tile([P, P], f32, tag="ysb")
                            nc.vector.tensor_scalar_mul(ysb[:], psy[j][:, 0:HD], recip[:])
                            pst = ps_t.tile([P, P], f32, tag="ps_t")
                            nc.tensor.transpose(pst[:], ysb[:], identity[:])
                            nc.vector.tensor_copy(YT[:, h, jg * P:(jg + 1) * P], pst[:])

                # ---- Phase 3: o_proj partial: out[t, e] = sum_h y_h^T.T @ woT_h
                for tb in range(TB):
                    for ec in range(4):
                        ps = ps_acc.tile([P, 512], f32, tag="ps_acc")
                        for h in range(NHC):
                            nc.tensor.matmul(
                                ps[:],
                                YT[:, h, tb * P:(tb + 1) * P],
                                WOT[:, h, ec * 512:(ec + 1) * 512],
                                start=(h == 0),
                                stop=(h == 3),
                            )
                        osb = owork.tile([P, 512], f32, tag="osb")
                        nc.vector.tensor_copy(osb[:], ps[:])
                        nc.sync.dma_start(out_r[:, tb, ec * 512:(ec + 1) * 512], osb[:])

    nc.finalize()
    return nc


def _get_nc():
    if "nc" not in _NC_CACHE:
        _NC_CACHE["nc"] = _build_nc()
    return _NC_CACHE["nc"]


def _in_maps(x, wq, wk, wv, wo):
    xTb = [np.ascontiguousarray(x[b].T) for b in range(B)]
    wqT = [np.ascontiguousarray(wq[g * 512:(g + 1) * 512].T) for g in range(NKV)]
    wkT = [np.ascontiguousarray(wk[g * HD:(g + 1) * HD].T) for g in range(NKV)]
    wvT = [np.ascontiguousarray(wv[g * HD:(g + 1) * HD].T) for g in range(NKV)]
    woT = [np.ascontiguousarray(wo[:, g * 512:(g + 1) * 512].T) for g in range(NKV)]
    maps = []
    for c in range(N_CORES):
        b, g = divmod(c, NKV)
        maps.append({
            "xT": xTb[b],
            "wqT": wqT[g],
            "wkT": wkT[g],
            "wvT": wvT[g],
            "woT": woT[g],
        })
    return maps


def kernel(x, wq, wk, wv, wo):
    from concourse.bass_utils import run_bass_kernel_spmd

    x = np.asarray(x, dtype=np.float32)
    wq = np.asarray(wq, dtype=np.float32)
    wk = np.asarray(wk, dtype=np.float32)
    wv = np.asarray(wv, dtype=np.float32)
    wo = np.asarray(wo, dtype=np.float32)

    nc = _get_nc()
    in_maps = _in_maps(x, wq, wk, wv, wo)

    res = run_bass_kernel_spmd(nc, in_maps, core_ids=list(range(N_CORES)))

    partials = [res.results[c]["out"] for c in range(N_CORES)]
    out = np.empty((B, T, E), dtype=np.float32)
    for b in range(B):
        acc = partials[NKV * b].astype(np.float32)
        for g in range(1, NKV):
            acc = acc + partials[NKV * b + g]
        out[b] = acc
    return out
